# revision 8
# baseline (speedup 1.0000x reference)
"""Causal depthwise conv1d with learnable hidden-state prefix, on 8 TRN2 cores.

Reference computation (per batch b, channel d):
    xp = concat([init_state[d, :3], x[b, d, :]])          # [L+3] = [4099]
    out[b, d, t] = bias[d] + sum_{j=0..3} w[d, j] * xp[t+j]   for t in [0, 4099)
    (xp index beyond 4098 contributes 0)

Sharding: channel dim D=4096 split 8 ways (512 channels/core), zero
communication. Each core processes rows (b, d_local) = 4*512 = 2048 rows of
length 4096 -> 16 SBUF tiles of [128 rows, full row].

I/O strategy: x streams in as fp32 (exact), the result streams out as fp16
(one final rounding of each output value: rel err ~2^-11, far inside the
tolerance) and is upcast to fp32 on the host. That puts the per-core DMA
floor at 32MB in + 16MB out ~= 140us @ 360GB/s.

Compute: all engines share the row so each stays under the DMA floor. Per
tile the columns split three ways:
  - PE chunks (exact fp32 matmuls, diagonal weight per tap, 4 taps
    accumulated in PSUM); ACT evacuates + bias -> fp16.  (float32r would be
    4x cheaper but is bf16-rounded on hardware -- fails the small-|y|
    relative-error floor.)
  - DVE region: ACT does tap0+bias into an fp32 scratch, DVE runs fused
    scalar*tensor+tensor MACs for taps 1-2, tap 3 writes fp16 directly.
  - Pool region: GPSIMD can't run scalar_tensor_tensor (walrus ISA check),
    so ACT forms each tap product via its free per-partition scale
    (tmp = w_j * x_shift) and Pool does plain tensor_tensor adds.
"""

import numpy as np

B, D, L = 4, 4096, 4096
KTAPS = 4
K = KTAPS - 1          # 3: state length
LOUT = L + K           # 4099
NCORES = 8
DSH = D // NCORES      # 512 channels per core
ROWS = B * DSH         # 2048 rows per core
P = 128                # SBUF partitions
NTILES = ROWS // P     # 16
G = DSH // P           # 4 channel groups per core

_CACHE = {}

MMCOLS = 512           # one PSUM bank of fp32 per matmul
PE_CHUNKS = (2,) * NTILES  # fp32 matmul chunks per tile
DVE_COLS = 2048            # DVE-region width; Pool gets the remainder


def _build_program(pe_chunks=PE_CHUNKS, dve_cols=DVE_COLS, in_bufs=5,
                   out_bufs=5, scr_bufs=2, tmp_bufs=2, split_in=(),
                   split_last_in=4, split_last_out=True):
    import concourse.bacc as bacc
    import concourse.mybir as mybir
    from concourse.tile import TileContext

    f32 = mybir.dt.float32
    f16 = mybir.dt.float16
    nc = bacc.Bacc("TRN2", target_bir_lowering=False, debug=False)

    xs = nc.dram_tensor("xs", [ROWS, L], f32, kind="ExternalInput").ap()
    # single packed param tensor -> single DMA -> single sync wait downstream.
    # layout per partition p: cols [g*4+j]=w[g*128+p, j] for g<4,j<4 (0..16),
    # col 16+g = bias[g*128+p], col 20+g*3+k = init_state[g*128+p, k]
    prm_d = nc.dram_tensor("prm", [P, 32], f32, kind="ExternalInput").ap()
    eye_d = nc.dram_tensor("eye", [P, P], f32, kind="ExternalInput").ap()
    out_d = nc.dram_tensor("out", [ROWS, LOUT], f16, kind="ExternalOutput").ap()

    with TileContext(nc) as tc:
        with (
            tc.tile_pool(name="consts", bufs=1) as cpool,
            tc.tile_pool(name="xin", bufs=in_bufs) as in_pool,
            tc.tile_pool(name="yout", bufs=out_bufs) as out_pool,
            tc.tile_pool(name="scr", bufs=scr_bufs) as scr_pool,
            tc.tile_pool(name="tmp", bufs=tmp_bufs) as tmp_pool,
            tc.tile_pool(name="psum", bufs=8, space="PSUM") as ps_pool,
        ):
            prm = cpool.tile([P, 32], f32)
            nc.sync.dma_start(out=prm, in_=prm_d)
            w_sb = prm[:, 0:G * KTAPS]
            b_sb = prm[:, 16:16 + G]
            s_sb = prm[:, 20:20 + G * K]

            # per-(group, tap) diagonal weight matrices for the PE path
            dg = {}
            if any(pe_chunks):
                eye = cpool.tile([P, P], f32)
                nc.sync.dma_start(out=eye, in_=eye_d)
                for g in range(G):
                    for j in range(KTAPS):
                        d = cpool.tile([P, P], f32, tag=f"diag{g}_{j}")
                        nc.vector.tensor_scalar_mul(
                            out=d, in0=eye,
                            scalar1=w_sb[:, g * KTAPS + j:g * KTAPS + j + 1])
                        dg[(g, j)] = d

            def dve_chain(out_t, scr, in_t, g, col0, n, scr0):
                """taps 1..3 for out cols [col0, col0+n) on DVE; tap j only
                reaches out col LOUT-j-1 (zero past x's end). The final tap
                writes fp16 into out_t; clipped columns finish in scr."""
                for j in range(1, KTAPS):
                    nj = min(n, LOUT - j - col0)
                    last = j == KTAPS - 1
                    nc.vector.scalar_tensor_tensor(
                        out=out_t[:, col0:col0 + nj] if last
                        else scr[:, scr0:scr0 + nj],
                        in0=in_t[:, 1 + j + col0:1 + j + col0 + nj],
                        scalar=w_sb[:, g * KTAPS + j:g * KTAPS + j + 1],
                        in1=scr[:, scr0:scr0 + nj],
                        op0=mybir.AluOpType.mult,
                        op1=mybir.AluOpType.add,
                    )
                nlast = min(n, LOUT - KTAPS + 1 - col0)
                if nlast < n:  # tail cols: all their taps landed in scr
                    nc.scalar.copy(out_t[:, col0 + nlast:col0 + n],
                                   scr[:, scr0 + nlast:scr0 + n])

            def pool_chain(out_t, scr, tmp, in_t, g, col0, n, scr0):
                """Same taps for the Pool region: ACT scales each shifted
                input by w_j (tmp), Pool accumulates with tensor_tensor
                adds; the final add writes fp16 into out_t."""
                for j in range(1, KTAPS):
                    nj = min(n, LOUT - j - col0)
                    tm = tmp[:, (j - 1) * n:(j - 1) * n + nj]
                    nc.scalar.activation(
                        tm, in_t[:, 1 + j + col0:1 + j + col0 + nj],
                        mybir.ActivationFunctionType.Identity,
                        bias=0.0,
                        scale=w_sb[:, g * KTAPS + j:g * KTAPS + j + 1])
                    last = j == KTAPS - 1
                    nc.gpsimd.tensor_tensor(
                        out=out_t[:, col0:col0 + nj] if last
                        else scr[:, scr0:scr0 + nj],
                        in0=scr[:, scr0:scr0 + nj], in1=tm,
                        op=mybir.AluOpType.add)
                nlast = min(n, LOUT - KTAPS + 1 - col0)
                if nlast < n:  # tail cols: all their taps landed in scr
                    nc.scalar.copy(out_t[:, col0 + nlast:col0 + n],
                                   scr[:, scr0 + nlast:scr0 + n])

            for t in range(NTILES):
                g = t % G  # channel group (tile order: batch-major)
                rows = slice(t * P, (t + 1) * P)
                ncols = pe_chunks[t] * MMCOLS   # PE-covered prefix
                nd = LOUT - ncols               # ACT tap0 + DVE/Pool suffix
                dn = min(dve_cols, nd)          # DVE subregion
                pn = nd - dn                    # Pool subregion

                # in_t: col 0 pad (16B align), state [1:4), x [4:4100)
                in_t = in_pool.tile([P, 1 + K + L], f32)
                npieces = split_last_in if t == NTILES - 1 else (
                    2 if t in split_in else 1)
                step = L // npieces
                for pc in range(npieces):
                    nc.sync.dma_start(
                        out=in_t[:, 1 + K + pc * step:1 + K + (pc + 1) * step],
                        in_=xs[rows, pc * step:(pc + 1) * step])
                nc.scalar.copy(in_t[:, 1:1 + K], s_sb[:, g * K:(g + 1) * K])

                out_t = out_pool.tile([P, LOUT], f16)
                # PE part: psum = sum_j diag(wj) @ in-shift, exact fp32;
                # ACT evacuates + adds bias, rounding once to fp16.
                for c in range(pe_chunks[t]):
                    ps = ps_pool.tile([P, MMCOLS], f32)
                    base = 1 + c * MMCOLS
                    for j in range(KTAPS):
                        nc.tensor.matmul(
                            ps, dg[(g, j)],
                            in_t[:, base + j:base + j + MMCOLS],
                            start=(j == 0), stop=(j == KTAPS - 1))
                    nc.scalar.activation(
                        out_t[:, c * MMCOLS:(c + 1) * MMCOLS], ps,
                        mybir.ActivationFunctionType.Identity,
                        bias=b_sb[:, g:g + 1], scale=1.0)

                # tap0 + bias for the whole DVE+Pool suffix, in one ACT op
                scr = scr_pool.tile([P, nd], f32)
                nc.scalar.activation(
                    scr, in_t[:, 1 + ncols:1 + LOUT],
                    mybir.ActivationFunctionType.Identity,
                    bias=b_sb[:, g:g + 1],
                    scale=w_sb[:, g * KTAPS:g * KTAPS + 1])
                dve_chain(out_t, scr, in_t, g, ncols, dn, 0)
                if pn:
                    tmp = tmp_pool.tile([P, (KTAPS - 1) * pn], f32)
                    pool_chain(out_t, scr, tmp, in_t, g, ncols + dn, pn, dn)

                # out-DMAs ride the Pool SWDGE ring: waits stall only the
                # Pool sequencer; both HWDGE rings stay wait-free.
                if t == NTILES - 1 and split_last_out:
                    # PE+DVE regions leave as soon as their writes land;
                    # only the Pool region trails (shorter drain tail).
                    nc.gpsimd.dma_start(out=out_d[rows, :ncols + dn],
                                        in_=out_t[:, :ncols + dn])
                    nc.gpsimd.dma_start(out=out_d[rows, ncols + dn:],
                                        in_=out_t[:, ncols + dn:])
                else:
                    nc.gpsimd.dma_start(out=out_d[rows, :], in_=out_t)

    nc.compile()
    return nc


def kernel(x, weight, bias, init_state):
    from concourse.bass_utils import run_bass_kernel_spmd

    assert x.shape == (B, D, L) and x.dtype == np.float32
    wl = np.ascontiguousarray(weight[:, 0, :], dtype=np.float32)      # [D, 4]
    bias = np.ascontiguousarray(bias, dtype=np.float32)               # [D]
    st = np.ascontiguousarray(init_state, dtype=np.float32)           # [D, 3]

    if "nc" not in _CACHE:
        _CACHE["nc"] = _build_program()
    nc = _CACHE["nc"]

    in_maps = []
    for c in range(NCORES):
        lo, hi = c * DSH, (c + 1) * DSH
        xs = np.ascontiguousarray(x[:, lo:hi, :]).reshape(ROWS, L)
        wc = wl[lo:hi]                                                # [512, 4]
        prm = np.zeros((P, 32), np.float32)
        prm[:, 0:G * KTAPS] = (
            wc.reshape(G, P, KTAPS).transpose(1, 0, 2).reshape(P, G * KTAPS))
        prm[:, 16:16 + G] = bias[lo:hi].reshape(G, P).T
        prm[:, 20:20 + G * K] = (
            st[lo:hi].reshape(G, P, K).transpose(1, 0, 2).reshape(P, G * K))
        in_maps.append({"xs": xs, "prm": prm,
                        "eye": np.eye(P, dtype=np.float32)})

    res = run_bass_kernel_spmd(nc, in_maps, core_ids=list(range(NCORES)))
    shards = [r["out"].reshape(B, DSH, LOUT) for r in res.results]
    return np.concatenate(shards, axis=1).astype(np.float32)
